# revision 1
# baseline (speedup 1.0000x reference)
"""Trainium2 Bass kernel for nn_Attention_28724741275862.

Reference computation (per batch b):
    dec_part[i,o] = dec[b] @ W_dec.T          # [64, 512]
    enc_part[j,o] = enc[b] @ W_enc.T          # [512, 512]
    logits[i,j,o] = dec_part[i,o] + enc_part[j,o] + bias[o]
    alpha = log_softmax(logits, axis=o)
    ctx[i,o] = sum_j alpha[i,j,o] * enc[b][j,o]

Factorization used here (exact in fp32, ~2e-3 rel err with bf16 operands):
    LSE[i,j] = log(sum_o exp(dec_part[i,o]) * exp(enc_part[j,o] + bias[o]))
             = log( (E_enc @ E_dec)[j,i] )            # a matmul over o!
    ctx[i,o] = dec_part[i,o]*S_enc[o] + C'[o] - (LSE @ enc[b])[i,o]
      S_enc[o] = sum_j enc[j,o]
      C'[o]    = sum_j (enc_part[j,o] + bias[o]) * enc[j,o]
               = C0'[o] + bias[o]*S_enc[o]

So the O(T_dec*T_enc*H2) log-softmax collapses into 4 matmuls + cheap
elementwise work. Sharding: data-parallel over batch B=8 across the 8
cores (encoderOutput/decoderInput sharded on dim 0, W/b replicated); no
collectives. Everything on-chip is computed in transposed layout
[feature_on_partitions, i_free] so per-feature broadcasts are
per-partition scalar operands.

Self-contained: hardcodes shapes B=8, T_dec=64, T_enc=512, H2=512.
"""

import sys

for _p in ("/opt/trn_rl_repo",):
    if _p not in sys.path:
        sys.path.insert(0, _p)

import numpy as np
import ml_dtypes

import concourse.bass as bass
import concourse.tile as tile
from concourse import bacc, mybir
from concourse.bass_utils import run_bass_kernel_spmd

B, T_DEC, T_ENC, H2 = 8, 64, 512, 512
P = 128  # SBUF partitions
NB = H2 // P  # 4 feature blocks

BF16 = mybir.dt.bfloat16
F32 = mybir.dt.float32
AF = mybir.ActivationFunctionType
ALU = mybir.AluOpType

_CACHE = {}

from contextlib import ExitStack

BA_W = H2 + T_DEC  # blkA row width


def build_raw(bacc, mybir, bass):
    BF16 = mybir.dt.bfloat16
    F32 = mybir.dt.float32
    AF = mybir.ActivationFunctionType
    ALU = mybir.AluOpType

    nc = bacc.Bacc(None, target_bir_lowering=False)

    encN = nc.dram_tensor("encN", [T_ENC, H2], BF16, kind="ExternalInput")
    a1blk = nc.dram_tensor("a1blk", [NB, 2, P, H2], BF16, kind="ExternalInput")
    blkA = nc.dram_tensor("blkA", [NB, P, BA_W], BF16, kind="ExternalInput")
    b4d = nc.dram_tensor("b4", [P, NB], F32, kind="ExternalInput")
    out = nc.dram_tensor("out", [H2, T_DEC], F32, kind="ExternalOutput")

    encN_r = encN[:, :].rearrange("(a p) o -> p a o", p=P)
    a1_r = a1blk[:, :, :, :].rearrange("a two p c -> p a two c")
    bA_r = blkA[:, :, :].rearrange("a p c -> p a c")
    out_r = out[:, :].rearrange("(a p) i -> p a i", p=P)

    with ExitStack() as ctx:
        ec = ctx.enter_context
        # ---- SBUF ----
        a1t = [ec(nc.sbuf_tensor(f"a1t{d}", [P, 2, H2], BF16)) for d in range(NB)]
        bt = ec(nc.sbuf_tensor("bt", [P, NB, BA_W], BF16))
        eN = ec(nc.sbuf_tensor("eN", [P, NB, H2], BF16))
        b4 = ec(nc.sbuf_tensor("b4s", [P, NB], F32))
        ee = [ec(nc.sbuf_tensor(f"ee{o}", [P, T_ENC], BF16)) for o in range(NB)]
        ed = [ec(nc.sbuf_tensor(f"ed{o}", [P, T_DEC], BF16)) for o in range(NB)]
        lt = [ec(nc.sbuf_tensor(f"lt{j}", [P, T_DEC + 1], BF16)) for j in range(NB)]
        junk = ec(nc.sbuf_tensor("junk", [P, NB, T_ENC], BF16))
        jbf = ec(nc.sbuf_tensor("jbf", [P, T_ENC], BF16))  # PE warmup junk
        wj = ec(nc.sbuf_tensor("wj", [P, NB], F32))
        cp = ec(nc.sbuf_tensor("cp", [P, NB], F32))
        se = ec(nc.sbuf_tensor("se", [P, NB], F32))
        fx = ec(nc.sbuf_tensor("fx", [P, NB], F32))
        ctmp = ec(nc.sbuf_tensor("ctmp", [P, NB, T_DEC], F32))
        ctxo = ec(nc.sbuf_tensor("ctxo", [P, NB, T_DEC], F32))
        # ---- PSUM (8 banks) ----
        pp = [ec(nc.psum_tensor(f"pp{o}", [P, T_ENC], F32)) for o in range(NB)]
        pd = ec(nc.psum_tensor("pd", [P, NB, T_DEC], F32))
        ps = ec(nc.psum_tensor("ps", [P, NB, T_DEC], F32))
        pcA = ec(nc.psum_tensor("pcA", [P, 2, T_DEC + 1], F32))
        pcB = ec(nc.psum_tensor("pcB", [P, 2, T_DEC + 1], F32))

        def pc(ob):
            return (pcA if ob < 2 else pcB)[:, ob % 2, :]

        def wte(db):
            return a1t[db][:, 0, :]

        def eT(db):
            return a1t[db][:, 1, :]

        jz = ec(nc.semaphore("jz"))
        dS = [ec(nc.semaphore(f"dS{d}")) for d in range(NB)]
        gA1 = ec(nc.semaphore("gA1"))
        gA2 = ec(nc.semaphore("gA2"))
        gA3 = ec(nc.semaphore("gA3"))
        dO = ec(nc.semaphore("dO"))
        pe = ec(nc.semaphore("pe"))
        ac = ec(nc.semaphore("ac"))
        dv = ec(nc.semaphore("dv"))

        with nc.Block(no_gpsimd_drain=True) as block:

            @block.sync
            def _(sync):
                for db in range(NB):
                    sync.dma_start(
                        out=a1t[db][:, :, :], in_=a1_r[:, db, :, :]
                    ).then_inc(dS[db], 16)
                sync.wait_ge(dv, 20)
                sync.dma_start(out=out_r[:, 0:2, :], in_=ctxo[:, 0:2, :]).then_inc(
                    dO, 16
                )
                sync.wait_ge(dv, 28)
                sync.dma_start(out=out_r[:, 2:NB, :], in_=ctxo[:, 2:NB, :]).then_inc(
                    dO, 16
                )
                sync.wait_ge(dO, 32)

            @block.gpsimd
            def _(gpsimd):
                gpsimd.dma_start(out=eN[:, :, :], in_=encN_r[:, :, :]).then_inc(
                    gA3, 16
                )

            @block.scalar
            def _(scalar):
                scalar.dma_start(out=bt[:, :, :], in_=bA_r[:, :, :]).then_inc(gA1, 16)
                scalar.dma_start(out=b4[:, :], in_=b4d[:, :]).then_inc(gA2, 16)
                # Exp warmup pulls the Exp table during the DMA phase. No Ln
                # warmup: the ACT table cache holds one function, so warming
                # Ln would force a 4th reload after the Exp block.
                scalar.activation(wj[:, 0:1], wj[:, 3:4], AF.Exp, scale=0.0).then_inc(
                    ac, 1
                )  # ac=1
                # E_dec = exp(dec_part^T); pd fully written at pe>=4
                scalar.wait_ge(pe, NB)
                for ob in range(NB):
                    scalar.activation(ed[ob][:, :], pd[:, ob, :], AF.Exp).then_inc(
                        ac, 1
                    )  # ac 2..5
                # E_enc = exp(enc_part^T + b)
                scalar.wait_ge(gA2, 16)
                for ob in range(NB):
                    scalar.wait_ge(pe, NB + 1 + ob)
                    scalar.activation(
                        ee[ob][:, :],
                        pp[ob][:, :],
                        AF.Exp,
                        bias=b4[:, ob : ob + 1],
                    ).then_inc(ac, 1)  # ac 6..9
                # Pull the Ln table load forward: after the last Exp there
                # are no more Exp ops, so this is still only the 2nd load,
                # but it overlaps the C matmuls instead of blocking the real
                # Lns. No then_inc -> ac ledger unchanged.
                scalar.activation(wj[:, 1:2], wj[:, 3:4], AF.Ln, bias=1.0, scale=0.0)
                # LSE^T = ln(S^T)
                scalar.wait_ge(pe, 3 * NB)
                for jb in range(NB):
                    scalar.activation(lt[jb][:, 0:T_DEC], ps[:, jb, :], AF.Ln).then_inc(
                        ac, 1
                    )  # ac 10..13

            @block.tensor
            def _(tensor):
                # HAM warmup on junk data (overwritten by A2's start=True)
                tensor.wait_ge(jz, 1)
                for k in range(10):
                    tensor.matmul(
                        pp[k % NB][:, :],
                        lhsT=jbf[:, 0:P],
                        rhs=jbf[:, :],
                        start=True,
                        stop=True,
                    )
                # A2 first (blkA on the scalar HWDGE ring lands first)
                tensor.wait_ge(gA1, 16)
                for ob in range(NB):
                    for db in range(NB):
                        mm = tensor.matmul(
                            pd[:, ob, :],
                            lhsT=bt[:, db, ob * P : (ob + 1) * P],
                            rhs=bt[:, db, H2 : H2 + T_DEC],
                            start=(db == 0),
                            stop=(db == NB - 1),
                        )
                        if db == NB - 1:
                            mm.then_inc(pe, 1)  # pe 1..4
                # A1
                for db in range(NB):
                    tensor.wait_ge(dS[db], 16)
                    for ob in range(NB):
                        mm = tensor.matmul(
                            pp[ob][:, :],
                            lhsT=wte(db)[:, ob * P : (ob + 1) * P],
                            rhs=eT(db)[:, :],
                            start=(db == 0),
                            stop=(db == NB - 1),
                        )
                        if db == NB - 1:
                            mm.then_inc(pe, 1)  # pe 5..8
                # C
                tensor.wait_ge(ac, 9)
                for jb in range(NB):
                    for ob in range(NB):
                        mm = tensor.matmul(
                            ps[:, jb, :],
                            lhsT=ee[ob][:, jb * P : (jb + 1) * P],
                            rhs=ed[ob][:, :],
                            start=(ob == 0),
                            stop=(ob == NB - 1),
                        )
                        if ob == NB - 1:
                            mm.then_inc(pe, 1)  # pe 9..12
                # E
                tensor.wait_ge(gA3, 16)  # eN
                tensor.wait_ge(dv, NB)  # lt ones columns
                for ob in range(NB):
                    for jb in range(NB):
                        if ob == 0:
                            tensor.wait_ge(ac, 9 + jb + 1)  # ln[jb]
                        mm = tensor.matmul(
                            pc(ob),
                            lhsT=eN[:, jb, ob * P : (ob + 1) * P],
                            rhs=lt[jb][:, :],
                            start=(jb == 0),
                            stop=(jb == NB - 1),
                        )
                        if jb == NB - 1:
                            mm.then_inc(pe, 1)  # pe 13..16

            @block.vector
            def _(vector):
                vector.memset(jbf[:, :], 0.0).then_inc(jz, 1)
                for jb in range(NB):
                    vector.memset(lt[jb][:, T_DEC : T_DEC + 1], 1.0).then_inc(
                        dv, 1
                    )  # dv 1..4
                # C0': ACT (ee) and DVE must not read the same PSUM bank
                # concurrently -> gate the mult on ee[ob] (ac 6..9)
                for ob in range(NB):
                    vector.wait_ge(ac, 6 + ob)
                    vector.tensor_tensor(
                        out=junk[:, ob, :],
                        in0=pp[ob][:, :],
                        in1=eT(ob)[:, :],
                        op=ALU.mult,
                    ).then_inc(dv, 1)  # dv 5+2ob
                    vector.wait_ge(dv, 5 + 2 * ob)
                    vector.reduce_sum(
                        out=cp[:, ob : ob + 1],
                        in_=junk[:, ob, :],
                        axis=mybir.AxisListType.X,
                    ).then_inc(dv, 1)  # dv 6+2ob
                vector.wait_ge(gA2, 16)  # b4 for fx
                for ob in range(NB):
                    vector.wait_ge(pe, 14 if ob < 2 else 16)
                    vector.tensor_copy(
                        se[:, ob : ob + 1], pc(ob)[:, T_DEC : T_DEC + 1]
                    ).then_inc(dv, 1)  # dv 13+4ob
                    vector.wait_ge(dv, 13 + 4 * ob)
                    vector.tensor_scalar(
                        out=fx[:, ob : ob + 1],
                        in0=se[:, ob : ob + 1],
                        scalar1=b4[:, ob : ob + 1],
                        scalar2=cp[:, ob : ob + 1],
                        op0=ALU.mult,
                        op1=ALU.add,
                    ).then_inc(dv, 1)  # dv 14+4ob
                    vector.wait_ge(dv, 14 + 4 * ob)
                    vector.tensor_scalar(
                        out=ctmp[:, ob, :],
                        in0=pd[:, ob, :],
                        scalar1=se[:, ob : ob + 1],
                        scalar2=fx[:, ob : ob + 1],
                        op0=ALU.mult,
                        op1=ALU.add,
                    ).then_inc(dv, 1)  # dv 15+4ob
                    vector.wait_ge(dv, 15 + 4 * ob)
                    vector.tensor_tensor(
                        out=ctxo[:, ob, :],
                        in0=ctmp[:, ob, :],
                        in1=pc(ob)[:, 0:T_DEC],
                        op=ALU.subtract,
                    ).then_inc(dv, 1)  # dv 16+4ob -> 28

        nc.finalize()
    return nc


def _build_nc():
    return build_raw(bacc, mybir, bass)


def _build_nc_tile():
    nc = bacc.Bacc(None, target_bir_lowering=False)

    # Per-core DRAM inputs (core = batch):
    #  encN [T_enc, H2] bf16   natural enc       (lhsT for ctx2 matmul)
    #  encT [H2, T_enc] bf16   transposed enc    (rhs for enc_part, C' term)
    #  decT [H2, T_dec] bf16   transposed dec    (rhs for dec_part)
    #  WT   [2*H2, H2]  bf16   W transposed; rows 0:H2 = W_dec^T, H2: = W_enc^T
    #  b4   [P, NB]     f32    bias swizzled per-partition: b4[p,k] = b[k*P+p]
    encN = nc.dram_tensor("encN", [T_ENC, H2], BF16, kind="ExternalInput")
    encT = nc.dram_tensor("encT", [H2, T_ENC], BF16, kind="ExternalInput")
    decT = nc.dram_tensor("decT", [H2, T_DEC], BF16, kind="ExternalInput")
    WT = nc.dram_tensor("WT", [2 * H2, H2], BF16, kind="ExternalInput")
    b4 = nc.dram_tensor("b4", [P, NB], F32, kind="ExternalInput")
    # out = ctx^T [H2, T_dec] f32; host transposes back.
    out = nc.dram_tensor("out", [H2, T_DEC], F32, kind="ExternalOutput")

    encN_r = encN[:, :].rearrange("(a p) o -> p a o", p=P)
    encT_r = encT[:, :].rearrange("(a p) j -> p a j", p=P)
    decT_r = decT[:, :].rearrange("(a p) i -> p a i", p=P)
    WT_r = WT[:, :].rearrange("(a p) o -> p a o", p=P)
    out_r = out[:, :].rearrange("(a p) i -> p a i", p=P)

    with tile.TileContext(nc) as tc:
        with (
            tc.tile_pool(name="ins", bufs=1) as ins,
            tc.tile_pool(name="mids", bufs=1) as mids,
            tc.tile_pool(name="ppool", bufs=1, space="PSUM") as ppool,
            tc.tile_pool(name="spool", bufs=1, space="PSUM") as spool,
        ):
            # ---- input DMAs (HWDGE) ----
            wte_t = [ins.tile([P, H2], BF16, name=f"wte{d}", tag=f"wte{d}") for d in range(NB)]
            eT_t = [ins.tile([P, T_ENC], BF16, name=f"eT{d}", tag=f"eT{d}") for d in range(NB)]
            for db in range(NB):
                nc.sync.dma_start(out=wte_t[db][:, :], in_=WT_r[:, NB + db, :])
                nc.sync.dma_start(out=eT_t[db][:, :], in_=encT_r[:, db, :])
            wtd_t = ins.tile([P, NB, H2], BF16)
            nc.sync.dma_start(out=wtd_t[:, :, :], in_=WT_r[:, 0:NB, :])
            dT_t = ins.tile([P, NB, T_DEC], BF16)
            nc.sync.dma_start(out=dT_t[:, :, :], in_=decT_r[:, :, :])
            b4_t = ins.tile([P, NB], F32)
            nc.sync.dma_start(out=b4_t[:, :], in_=b4[:, :])
            eN_t = ins.tile([P, NB, H2], BF16)
            nc.sync.dma_start(out=eN_t[:, :, :], in_=encN_r[:, :, :])

            # ---- A1: enc_part^T[o, j] += W_enc^T[d, o].T @ enc^T[d, j] ----
            pp = [ppool.tile([P, T_ENC], F32, name=f"pp{o}", tag=f"pp{o}") for o in range(NB)]
            for db in range(NB):
                for ob in range(NB):
                    nc.tensor.matmul(
                        pp[ob][:, :],
                        lhsT=wte_t[db][:, ob * P : (ob + 1) * P],
                        rhs=eT_t[db][:, :],
                        start=(db == 0),
                        stop=(db == NB - 1),
                    )

            # ---- A2: dec_part^T[o, i] += W_dec^T[d, o].T @ dec^T[d, i] ----
            pd = spool.tile([P, NB, T_DEC], F32, name="pdall")
            for ob in range(NB):
                for db in range(NB):
                    nc.tensor.matmul(
                        pd[:, ob, :],
                        lhsT=wtd_t[:, db, ob * P : (ob + 1) * P],
                        rhs=dT_t[:, db, :],
                        start=(db == 0),
                        stop=(db == NB - 1),
                    )

            # ---- B: exponentials (ACT), keep dec_part, C' partial (DVE) ----
            ee_t = [mids.tile([P, T_ENC], BF16, name=f"ee{o}", tag=f"ee{o}") for o in range(NB)]
            ed_t = [mids.tile([P, T_DEC], BF16, name=f"ed{o}", tag=f"ed{o}") for o in range(NB)]
            dp_t = [mids.tile([P, T_DEC], F32, name=f"dp{o}", tag=f"dp{o}") for o in range(NB)]
            cp_t = mids.tile([P, NB], F32)  # C0' per feature block
            junk = mids.tile([P, T_ENC], F32)  # ttr elementwise product sink
            for ob in range(NB):
                # E_enc^T = exp(enc_part^T + bias)
                nc.scalar.activation(
                    ee_t[ob][:, :],
                    pp[ob][:, :],
                    AF.Exp,
                    bias=b4_t[:, ob : ob + 1],
                )
                # E_dec^T = exp(dec_part^T)
                nc.scalar.activation(ed_t[ob][:, :], pd[:, ob, :], AF.Exp)
                # keep dec_part^T for the final combine
                nc.vector.tensor_copy(dp_t[ob][:, :], pd[:, ob, :])
                # C0'[o] = sum_j enc_part^T[o,j] * enc^T[o,j]
                # (tensor_tensor_reduce NEFFs fail at runtime here; use
                # separate mult + reduce)
                nc.vector.tensor_tensor(
                    out=junk[:, :],
                    in0=pp[ob][:, :],
                    in1=eT_t[ob][:, :],
                    op=ALU.mult,
                )
                nc.vector.reduce_sum(
                    out=cp_t[:, ob : ob + 1],
                    in_=junk[:, :],
                    axis=mybir.AxisListType.X,
                )

            # ---- C: S^T[j, i] += E_enc^T[o, j].T @ E_dec^T[o, i] ----
            ps = spool.tile([P, NB, T_DEC], F32, name="psall")
            for jb in range(NB):
                for ob in range(NB):
                    nc.tensor.matmul(
                        ps[:, jb, :],
                        lhsT=ee_t[ob][:, jb * P : (jb + 1) * P],
                        rhs=ed_t[ob][:, :],
                        start=(ob == 0),
                        stop=(ob == NB - 1),
                    )

            # ---- D: LSE^T = ln(S^T), with a ones column for S_enc ----
            lt_t = [mids.tile([P, T_DEC + 1], BF16, name=f"lt{j}", tag=f"lt{j}") for j in range(NB)]
            for jb in range(NB):
                nc.vector.memset(lt_t[jb][:, T_DEC : T_DEC + 1], 1.0)
                nc.scalar.activation(lt_t[jb][:, 0:T_DEC], ps[:, jb, :], AF.Ln)

            # ---- E: [ctx2^T | S_enc][o, :] += enc[j, o].T @ [LSE^T | 1] ----
            pc = spool.tile([P, NB, T_DEC + 1], F32, name="pcall")
            for ob in range(NB):
                for jb in range(NB):
                    nc.tensor.matmul(
                        pc[:, ob, :],
                        lhsT=eN_t[:, jb, ob * P : (ob + 1) * P],
                        rhs=lt_t[jb][:, :],
                        start=(jb == 0),
                        stop=(jb == NB - 1),
                    )

            # ---- G: ctx^T = dec_part^T * S_enc + (b*S_enc + C0') - ctx2^T ----
            se_t = mids.tile([P, NB], F32)
            fix_t = mids.tile([P, NB], F32)
            ctxo = mids.tile([P, NB, T_DEC], F32)
            for ob in range(NB):
                nc.vector.tensor_copy(se_t[:, ob : ob + 1], pc[:, ob, T_DEC : T_DEC + 1])
                # fix = b*S_enc + C0'
                nc.vector.tensor_scalar(
                    out=fix_t[:, ob : ob + 1],
                    in0=se_t[:, ob : ob + 1],
                    scalar1=b4_t[:, ob : ob + 1],
                    scalar2=cp_t[:, ob : ob + 1],
                    op0=ALU.mult,
                    op1=ALU.add,
                )
                # ctx = dp*S_enc + fix
                nc.vector.tensor_scalar(
                    out=ctxo[:, ob, :],
                    in0=dp_t[ob][:, :],
                    scalar1=se_t[:, ob : ob + 1],
                    scalar2=fix_t[:, ob : ob + 1],
                    op0=ALU.mult,
                    op1=ALU.add,
                )
                # ctx -= ctx2
                nc.vector.tensor_tensor(
                    out=ctxo[:, ob, :],
                    in0=ctxo[:, ob, :],
                    in1=pc[:, ob, 0:T_DEC],
                    op=ALU.subtract,
                )
            nc.sync.dma_start(out=out_r[:, :, :], in_=ctxo[:, :, :])

    nc.finalize()
    return nc


def _prep_in_maps(encoderOutput, decoderInput, W, b):
    bf = ml_dtypes.bfloat16
    WT = np.ascontiguousarray(np.asarray(W, np.float32).T)  # [2H, H]
    b4 = np.ascontiguousarray(np.asarray(b, np.float32).reshape(NB, P).T)
    in_maps = []
    for core in range(B):
        e = np.asarray(encoderOutput[core], np.float32)
        d = np.asarray(decoderInput[core], np.float32)
        eT = e.T  # [H2, T_enc]
        dT = d.T  # [H2, T_dec]
        # a1blk[db, 0] = W_enc^T rows db-block; a1blk[db, 1] = encT rows
        a1 = np.empty((NB, 2, P, H2), np.float32)
        a1[:, 0] = WT[H2:].reshape(NB, P, H2)
        a1[:, 1] = eT.reshape(NB, P, T_ENC)
        # blkA[db, p] = [W_dec^T row | decT row]
        bA = np.empty((NB, P, H2 + T_DEC), np.float32)
        bA[:, :, :H2] = WT[:H2].reshape(NB, P, H2)
        bA[:, :, H2:] = dT.reshape(NB, P, T_DEC)
        in_maps.append(
            {
                "encN": e.astype(bf),
                "a1blk": a1.astype(bf),
                "blkA": bA.astype(bf),
                "b4": b4,
            }
        )
    return in_maps


def kernel(encoderOutput, decoderInput, W, b, _trace=False):
    if "nc" not in _CACHE:
        _CACHE["nc"] = _build_nc()
    nc = _CACHE["nc"]
    in_maps = _prep_in_maps(encoderOutput, decoderInput, W, b)
    res = run_bass_kernel_spmd(nc, in_maps, core_ids=list(range(B)), trace=_trace)
    outs = np.stack([np.asarray(r["out"], np.float32).T for r in res.results])
    if _trace:
        _CACHE["last_result"] = res
    return outs



# revision 12
# speedup vs baseline: 1.1818x; 1.1818x over previous
"""Trainium2 Bass kernel for nn_Attention_28724741275862.

Reference computation (per batch b):
    dec_part[i,o] = dec[b] @ W_dec.T          # [64, 512]
    enc_part[j,o] = enc[b] @ W_enc.T          # [512, 512]
    logits[i,j,o] = dec_part[i,o] + enc_part[j,o] + bias[o]
    alpha = log_softmax(logits, axis=o)
    ctx[i,o] = sum_j alpha[i,j,o] * enc[b][j,o]

Factorization used here (exact in fp32, ~2e-3 rel err with bf16 operands):
    LSE[i,j] = log(sum_o exp(dec_part[i,o]) * exp(enc_part[j,o] + bias[o]))
             = log( (E_enc @ E_dec)[j,i] )            # a matmul over o!
    ctx[i,o] = dec_part[i,o]*S_enc[o] + C'[o] - (LSE @ enc[b])[i,o]
      S_enc[o] = sum_j enc[j,o]
      C'[o]    = sum_j (enc_part[j,o] + bias[o]) * enc[j,o]
               = C0'[o] + bias[o]*S_enc[o]

So the O(T_dec*T_enc*H2) log-softmax collapses into 4 matmuls + cheap
elementwise work. Sharding: data-parallel over batch B=8 across the 8
cores (encoderOutput/decoderInput sharded on dim 0, W/b replicated); no
collectives. Everything on-chip is computed in transposed layout
[feature_on_partitions, i_free] so per-feature broadcasts are
per-partition scalar operands.

Self-contained: hardcodes shapes B=8, T_dec=64, T_enc=512, H2=512.
"""

import sys

for _p in ("/opt/trn_rl_repo",):
    if _p not in sys.path:
        sys.path.insert(0, _p)

import numpy as np
import ml_dtypes

import concourse.bass as bass
import concourse.tile as tile
from concourse import bacc, mybir
from concourse.bass_utils import run_bass_kernel_spmd

B, T_DEC, T_ENC, H2 = 8, 64, 512, 512
P = 128  # SBUF partitions
NB = H2 // P  # 4 feature blocks

BF16 = mybir.dt.bfloat16
F32 = mybir.dt.float32
AF = mybir.ActivationFunctionType
ALU = mybir.AluOpType

_CACHE = {}

from contextlib import ExitStack

BA_W = H2 + T_DEC  # blkA row width


def build_raw(bacc, mybir, bass):
    BF16 = mybir.dt.bfloat16
    F32 = mybir.dt.float32
    AF = mybir.ActivationFunctionType
    ALU = mybir.AluOpType

    nc = bacc.Bacc(None, target_bir_lowering=False)

    encN = nc.dram_tensor("encN", [T_ENC, H2], BF16, kind="ExternalInput")
    a1blk = nc.dram_tensor("a1blk", [NB, 2, P, H2], BF16, kind="ExternalInput")
    blkA = nc.dram_tensor("blkA", [NB, P, BA_W], BF16, kind="ExternalInput")
    b4d = nc.dram_tensor("b4", [P, NB], F32, kind="ExternalInput")
    out = nc.dram_tensor("out", [H2, T_DEC], F32, kind="ExternalOutput")

    encN_r = encN[:, :].rearrange("(a p) o -> p a o", p=P)
    a1_r = a1blk[:, :, :, :].rearrange("a two p c -> p a two c")
    bA_r = blkA[:, :, :].rearrange("a p c -> p a c")
    out_r = out[:, :].rearrange("(a p) i -> p a i", p=P)

    with ExitStack() as ctx:
        ec = ctx.enter_context
        # ---- SBUF ----
        a1t = [ec(nc.sbuf_tensor(f"a1t{d}", [P, 2, H2], BF16)) for d in range(NB)]
        bt = ec(nc.sbuf_tensor("bt", [P, NB, BA_W], BF16))
        eN = ec(nc.sbuf_tensor("eN", [P, NB, H2], BF16))
        b4 = ec(nc.sbuf_tensor("b4s", [P, NB], F32))
        ee = [ec(nc.sbuf_tensor(f"ee{o}", [P, T_ENC], BF16)) for o in range(NB)]
        ed = [ec(nc.sbuf_tensor(f"ed{o}", [P, T_DEC], BF16)) for o in range(NB)]
        lt = [ec(nc.sbuf_tensor(f"lt{j}", [P, T_DEC + 1], BF16)) for j in range(NB)]
        junk = ec(nc.sbuf_tensor("junk", [P, NB, T_ENC], BF16))
        jbf = ec(nc.sbuf_tensor("jbf", [P, T_ENC], BF16))  # PE warmup junk
        wj = ec(nc.sbuf_tensor("wj", [P, NB], F32))
        cp = ec(nc.sbuf_tensor("cp", [P, NB], F32))
        se = ec(nc.sbuf_tensor("se", [P, NB], F32))
        fx = ec(nc.sbuf_tensor("fx", [P, NB], F32))
        ctmp = ec(nc.sbuf_tensor("ctmp", [P, NB, T_DEC], F32))
        ctxo = ec(nc.sbuf_tensor("ctxo", [P, NB, T_DEC], F32))
        # ---- PSUM (8 banks) ----
        pp = [ec(nc.psum_tensor(f"pp{o}", [P, T_ENC], F32)) for o in range(NB)]
        pd = ec(nc.psum_tensor("pd", [P, NB, T_DEC], F32))
        ps = ec(nc.psum_tensor("ps", [P, NB, T_DEC], F32))
        pcA = ec(nc.psum_tensor("pcA", [P, 2, T_DEC + 1], F32))
        pcB = ec(nc.psum_tensor("pcB", [P, 2, T_DEC + 1], F32))

        def pc(ob):
            return (pcA if ob < 2 else pcB)[:, ob % 2, :]

        def wte(db):
            return a1t[db][:, 0, :]

        def eT(db):
            return a1t[db][:, 1, :]

        jz = ec(nc.semaphore("jz"))
        dS = [ec(nc.semaphore(f"dS{d}")) for d in range(NB)]
        gA1 = ec(nc.semaphore("gA1"))
        gA2 = ec(nc.semaphore("gA2"))
        gA3 = ec(nc.semaphore("gA3"))
        dO = ec(nc.semaphore("dO"))
        pe = ec(nc.semaphore("pe"))
        ac = ec(nc.semaphore("ac"))
        dv = ec(nc.semaphore("dv"))

        with nc.Block(no_gpsimd_drain=True) as block:

            @block.sync
            def _(sync):
                for db in range(NB):
                    sync.dma_start(
                        out=a1t[db][:, :, :], in_=a1_r[:, db, :, :]
                    ).then_inc(dS[db], 16)
                sync.wait_ge(dv, 20)
                sync.dma_start(out=out_r[:, 0:2, :], in_=ctxo[:, 0:2, :]).then_inc(
                    dO, 16
                )
                sync.wait_ge(dv, 28)
                sync.dma_start(out=out_r[:, 2:NB, :], in_=ctxo[:, 2:NB, :]).then_inc(
                    dO, 16
                )
                sync.wait_ge(dO, 32)

            @block.gpsimd
            def _(gpsimd):
                gpsimd.dma_start(out=eN[:, :, :], in_=encN_r[:, :, :]).then_inc(
                    gA3, 16
                )

            @block.scalar
            def _(scalar):
                scalar.dma_start(out=bt[:, :, :], in_=bA_r[:, :, :]).then_inc(gA1, 16)
                scalar.dma_start(out=b4[:, :], in_=b4d[:, :]).then_inc(gA2, 16)
                # Exp warmup pulls the Exp table during the DMA phase. No Ln
                # warmup: the ACT table cache holds one function, so warming
                # Ln would force a 4th reload after the Exp block.
                scalar.activation(wj[:, 0:1], wj[:, 3:4], AF.Exp, scale=0.0).then_inc(
                    ac, 1
                )  # ac=1
                # E_dec = exp(dec_part^T); pd fully written at pe>=4
                scalar.wait_ge(pe, NB)
                for ob in range(NB):
                    scalar.activation(ed[ob][:, :], pd[:, ob, :], AF.Exp).then_inc(
                        ac, 1
                    )  # ac 2..5
                # E_enc = exp(enc_part^T + b)
                scalar.wait_ge(gA2, 16)
                for ob in range(NB):
                    scalar.wait_ge(pe, NB + 1 + ob)
                    scalar.activation(
                        ee[ob][:, :],
                        pp[ob][:, :],
                        AF.Exp,
                        bias=b4[:, ob : ob + 1],
                    ).then_inc(ac, 1)  # ac 6..9
                # Pull the Ln table load forward: after the last Exp there
                # are no more Exp ops, so this is still only the 2nd load,
                # but it overlaps the C matmuls instead of blocking the real
                # Lns. No then_inc -> ac ledger unchanged.
                scalar.activation(wj[:, 1:2], wj[:, 3:4], AF.Ln, bias=1.0, scale=0.0)
                # LSE^T = ln(S^T)
                scalar.wait_ge(pe, 3 * NB)
                for jb in range(NB):
                    scalar.activation(lt[jb][:, 0:T_DEC], ps[:, jb, :], AF.Ln).then_inc(
                        ac, 1
                    )  # ac 10..13

            @block.tensor
            def _(tensor):
                # HAM warmup on junk data (overwritten by A2's start=True)
                tensor.wait_ge(jz, 1)
                for k in range(10):
                    tensor.matmul(
                        pp[k % NB][:, :],
                        lhsT=jbf[:, 0:P],
                        rhs=jbf[:, :],
                        start=True,
                        stop=True,
                    )
                # A2 first (blkA on the scalar HWDGE ring lands first)
                tensor.wait_ge(gA1, 16)
                for ob in range(NB):
                    for db in range(NB):
                        mm = tensor.matmul(
                            pd[:, ob, :],
                            lhsT=bt[:, db, ob * P : (ob + 1) * P],
                            rhs=bt[:, db, H2 : H2 + T_DEC],
                            start=(db == 0),
                            stop=(db == NB - 1),
                        )
                        if db == NB - 1:
                            mm.then_inc(pe, 1)  # pe 1..4
                # A1
                for db in range(NB):
                    tensor.wait_ge(dS[db], 16)
                    for ob in range(NB):
                        mm = tensor.matmul(
                            pp[ob][:, :],
                            lhsT=wte(db)[:, ob * P : (ob + 1) * P],
                            rhs=eT(db)[:, :],
                            start=(db == 0),
                            stop=(db == NB - 1),
                        )
                        if db == NB - 1:
                            mm.then_inc(pe, 1)  # pe 5..8
                # C
                tensor.wait_ge(ac, 9)
                for jb in range(NB):
                    for ob in range(NB):
                        mm = tensor.matmul(
                            ps[:, jb, :],
                            lhsT=ee[ob][:, jb * P : (jb + 1) * P],
                            rhs=ed[ob][:, :],
                            start=(ob == 0),
                            stop=(ob == NB - 1),
                        )
                        if ob == NB - 1:
                            mm.then_inc(pe, 1)  # pe 9..12
                # E
                tensor.wait_ge(gA3, 16)  # eN
                tensor.wait_ge(dv, NB)  # lt ones columns
                for ob in range(NB):
                    for jb in range(NB):
                        if ob == 0:
                            tensor.wait_ge(ac, 9 + jb + 1)  # ln[jb]
                        mm = tensor.matmul(
                            pc(ob),
                            lhsT=eN[:, jb, ob * P : (ob + 1) * P],
                            rhs=lt[jb][:, :],
                            start=(jb == 0),
                            stop=(jb == NB - 1),
                        )
                        if jb == NB - 1:
                            mm.then_inc(pe, 1)  # pe 13..16

            @block.vector
            def _(vector):
                vector.memset(jbf[:, :], 0.0).then_inc(jz, 1)
                for jb in range(NB):
                    vector.memset(lt[jb][:, T_DEC : T_DEC + 1], 1.0).then_inc(
                        dv, 1
                    )  # dv 1..4
                # C0': ACT (ee) and DVE must not read the same PSUM bank
                # concurrently -> gate the mult on ee[ob] (ac 6..9)
                for ob in range(NB):
                    vector.wait_ge(ac, 6 + ob)
                    vector.tensor_tensor(
                        out=junk[:, ob, :],
                        in0=pp[ob][:, :],
                        in1=eT(ob)[:, :],
                        op=ALU.mult,
                    ).then_inc(dv, 1)  # dv 5+2ob
                    vector.wait_ge(dv, 5 + 2 * ob)
                    vector.reduce_sum(
                        out=cp[:, ob : ob + 1],
                        in_=junk[:, ob, :],
                        axis=mybir.AxisListType.X,
                    ).then_inc(dv, 1)  # dv 6+2ob
                vector.wait_ge(gA2, 16)  # b4 for fx
                for ob in range(NB):
                    vector.wait_ge(pe, 14 if ob < 2 else 16)
                    vector.tensor_copy(
                        se[:, ob : ob + 1], pc(ob)[:, T_DEC : T_DEC + 1]
                    ).then_inc(dv, 1)  # dv 13+4ob
                    vector.wait_ge(dv, 13 + 4 * ob)
                    vector.tensor_scalar(
                        out=fx[:, ob : ob + 1],
                        in0=se[:, ob : ob + 1],
                        scalar1=b4[:, ob : ob + 1],
                        scalar2=cp[:, ob : ob + 1],
                        op0=ALU.mult,
                        op1=ALU.add,
                    ).then_inc(dv, 1)  # dv 14+4ob
                    vector.wait_ge(dv, 14 + 4 * ob)
                    vector.tensor_scalar(
                        out=ctmp[:, ob, :],
                        in0=pd[:, ob, :],
                        scalar1=se[:, ob : ob + 1],
                        scalar2=fx[:, ob : ob + 1],
                        op0=ALU.mult,
                        op1=ALU.add,
                    ).then_inc(dv, 1)  # dv 15+4ob
                    vector.wait_ge(dv, 15 + 4 * ob)
                    vector.tensor_tensor(
                        out=ctxo[:, ob, :],
                        in0=ctmp[:, ob, :],
                        in1=pc(ob)[:, 0:T_DEC],
                        op=ALU.subtract,
                    ).then_inc(dv, 1)  # dv 16+4ob -> 28

        nc.finalize()
    return nc


def _build_nc():
    return build_raw(bacc, mybir, bass)


def _build_nc_tile():
    nc = bacc.Bacc(None, target_bir_lowering=False)

    # Per-core DRAM inputs (core = batch):
    #  encN [T_enc, H2] bf16   natural enc       (lhsT for ctx2 matmul)
    #  encT [H2, T_enc] bf16   transposed enc    (rhs for enc_part, C' term)
    #  decT [H2, T_dec] bf16   transposed dec    (rhs for dec_part)
    #  WT   [2*H2, H2]  bf16   W transposed; rows 0:H2 = W_dec^T, H2: = W_enc^T
    #  b4   [P, NB]     f32    bias swizzled per-partition: b4[p,k] = b[k*P+p]
    encN = nc.dram_tensor("encN", [T_ENC, H2], BF16, kind="ExternalInput")
    encT = nc.dram_tensor("encT", [H2, T_ENC], BF16, kind="ExternalInput")
    decT = nc.dram_tensor("decT", [H2, T_DEC], BF16, kind="ExternalInput")
    WT = nc.dram_tensor("WT", [2 * H2, H2], BF16, kind="ExternalInput")
    b4 = nc.dram_tensor("b4", [P, NB], F32, kind="ExternalInput")
    # out = ctx^T [H2, T_dec] f32; host transposes back.
    out = nc.dram_tensor("out", [H2, T_DEC], F32, kind="ExternalOutput")

    encN_r = encN[:, :].rearrange("(a p) o -> p a o", p=P)
    encT_r = encT[:, :].rearrange("(a p) j -> p a j", p=P)
    decT_r = decT[:, :].rearrange("(a p) i -> p a i", p=P)
    WT_r = WT[:, :].rearrange("(a p) o -> p a o", p=P)
    out_r = out[:, :].rearrange("(a p) i -> p a i", p=P)

    with tile.TileContext(nc) as tc:
        with (
            tc.tile_pool(name="ins", bufs=1) as ins,
            tc.tile_pool(name="mids", bufs=1) as mids,
            tc.tile_pool(name="ppool", bufs=1, space="PSUM") as ppool,
            tc.tile_pool(name="spool", bufs=1, space="PSUM") as spool,
        ):
            # ---- input DMAs (HWDGE) ----
            wte_t = [ins.tile([P, H2], BF16, name=f"wte{d}", tag=f"wte{d}") for d in range(NB)]
            eT_t = [ins.tile([P, T_ENC], BF16, name=f"eT{d}", tag=f"eT{d}") for d in range(NB)]
            for db in range(NB):
                nc.sync.dma_start(out=wte_t[db][:, :], in_=WT_r[:, NB + db, :])
                nc.sync.dma_start(out=eT_t[db][:, :], in_=encT_r[:, db, :])
            wtd_t = ins.tile([P, NB, H2], BF16)
            nc.sync.dma_start(out=wtd_t[:, :, :], in_=WT_r[:, 0:NB, :])
            dT_t = ins.tile([P, NB, T_DEC], BF16)
            nc.sync.dma_start(out=dT_t[:, :, :], in_=decT_r[:, :, :])
            b4_t = ins.tile([P, NB], F32)
            nc.sync.dma_start(out=b4_t[:, :], in_=b4[:, :])
            eN_t = ins.tile([P, NB, H2], BF16)
            nc.sync.dma_start(out=eN_t[:, :, :], in_=encN_r[:, :, :])

            # ---- A1: enc_part^T[o, j] += W_enc^T[d, o].T @ enc^T[d, j] ----
            pp = [ppool.tile([P, T_ENC], F32, name=f"pp{o}", tag=f"pp{o}") for o in range(NB)]
            for db in range(NB):
                for ob in range(NB):
                    nc.tensor.matmul(
                        pp[ob][:, :],
                        lhsT=wte_t[db][:, ob * P : (ob + 1) * P],
                        rhs=eT_t[db][:, :],
                        start=(db == 0),
                        stop=(db == NB - 1),
                    )

            # ---- A2: dec_part^T[o, i] += W_dec^T[d, o].T @ dec^T[d, i] ----
            pd = spool.tile([P, NB, T_DEC], F32, name="pdall")
            for ob in range(NB):
                for db in range(NB):
                    nc.tensor.matmul(
                        pd[:, ob, :],
                        lhsT=wtd_t[:, db, ob * P : (ob + 1) * P],
                        rhs=dT_t[:, db, :],
                        start=(db == 0),
                        stop=(db == NB - 1),
                    )

            # ---- B: exponentials (ACT), keep dec_part, C' partial (DVE) ----
            ee_t = [mids.tile([P, T_ENC], BF16, name=f"ee{o}", tag=f"ee{o}") for o in range(NB)]
            ed_t = [mids.tile([P, T_DEC], BF16, name=f"ed{o}", tag=f"ed{o}") for o in range(NB)]
            dp_t = [mids.tile([P, T_DEC], F32, name=f"dp{o}", tag=f"dp{o}") for o in range(NB)]
            cp_t = mids.tile([P, NB], F32)  # C0' per feature block
            junk = mids.tile([P, T_ENC], F32)  # ttr elementwise product sink
            for ob in range(NB):
                # E_enc^T = exp(enc_part^T + bias)
                nc.scalar.activation(
                    ee_t[ob][:, :],
                    pp[ob][:, :],
                    AF.Exp,
                    bias=b4_t[:, ob : ob + 1],
                )
                # E_dec^T = exp(dec_part^T)
                nc.scalar.activation(ed_t[ob][:, :], pd[:, ob, :], AF.Exp)
                # keep dec_part^T for the final combine
                nc.vector.tensor_copy(dp_t[ob][:, :], pd[:, ob, :])
                # C0'[o] = sum_j enc_part^T[o,j] * enc^T[o,j]
                # (tensor_tensor_reduce NEFFs fail at runtime here; use
                # separate mult + reduce)
                nc.vector.tensor_tensor(
                    out=junk[:, :],
                    in0=pp[ob][:, :],
                    in1=eT_t[ob][:, :],
                    op=ALU.mult,
                )
                nc.vector.reduce_sum(
                    out=cp_t[:, ob : ob + 1],
                    in_=junk[:, :],
                    axis=mybir.AxisListType.X,
                )

            # ---- C: S^T[j, i] += E_enc^T[o, j].T @ E_dec^T[o, i] ----
            ps = spool.tile([P, NB, T_DEC], F32, name="psall")
            for jb in range(NB):
                for ob in range(NB):
                    nc.tensor.matmul(
                        ps[:, jb, :],
                        lhsT=ee_t[ob][:, jb * P : (jb + 1) * P],
                        rhs=ed_t[ob][:, :],
                        start=(ob == 0),
                        stop=(ob == NB - 1),
                    )

            # ---- D: LSE^T = ln(S^T), with a ones column for S_enc ----
            lt_t = [mids.tile([P, T_DEC + 1], BF16, name=f"lt{j}", tag=f"lt{j}") for j in range(NB)]
            for jb in range(NB):
                nc.vector.memset(lt_t[jb][:, T_DEC : T_DEC + 1], 1.0)
                nc.scalar.activation(lt_t[jb][:, 0:T_DEC], ps[:, jb, :], AF.Ln)

            # ---- E: [ctx2^T | S_enc][o, :] += enc[j, o].T @ [LSE^T | 1] ----
            pc = spool.tile([P, NB, T_DEC + 1], F32, name="pcall")
            for ob in range(NB):
                for jb in range(NB):
                    nc.tensor.matmul(
                        pc[:, ob, :],
                        lhsT=eN_t[:, jb, ob * P : (ob + 1) * P],
                        rhs=lt_t[jb][:, :],
                        start=(jb == 0),
                        stop=(jb == NB - 1),
                    )

            # ---- G: ctx^T = dec_part^T * S_enc + (b*S_enc + C0') - ctx2^T ----
            se_t = mids.tile([P, NB], F32)
            fix_t = mids.tile([P, NB], F32)
            ctxo = mids.tile([P, NB, T_DEC], F32)
            for ob in range(NB):
                nc.vector.tensor_copy(se_t[:, ob : ob + 1], pc[:, ob, T_DEC : T_DEC + 1])
                # fix = b*S_enc + C0'
                nc.vector.tensor_scalar(
                    out=fix_t[:, ob : ob + 1],
                    in0=se_t[:, ob : ob + 1],
                    scalar1=b4_t[:, ob : ob + 1],
                    scalar2=cp_t[:, ob : ob + 1],
                    op0=ALU.mult,
                    op1=ALU.add,
                )
                # ctx = dp*S_enc + fix
                nc.vector.tensor_scalar(
                    out=ctxo[:, ob, :],
                    in0=dp_t[ob][:, :],
                    scalar1=se_t[:, ob : ob + 1],
                    scalar2=fix_t[:, ob : ob + 1],
                    op0=ALU.mult,
                    op1=ALU.add,
                )
                # ctx -= ctx2
                nc.vector.tensor_tensor(
                    out=ctxo[:, ob, :],
                    in0=ctxo[:, ob, :],
                    in1=pc[:, ob, 0:T_DEC],
                    op=ALU.subtract,
                )
            nc.sync.dma_start(out=out_r[:, :, :], in_=ctxo[:, :, :])

    nc.finalize()
    return nc


def _prep_in_maps(encoderOutput, decoderInput, W, b):
    bf = ml_dtypes.bfloat16
    WT = np.ascontiguousarray(np.asarray(W, np.float32).T)  # [2H, H]
    b4 = np.ascontiguousarray(np.asarray(b, np.float32).reshape(NB, P).T)
    in_maps = []
    for core in range(B):
        e = np.asarray(encoderOutput[core], np.float32)
        d = np.asarray(decoderInput[core], np.float32)
        eT = e.T  # [H2, T_enc]
        dT = d.T  # [H2, T_dec]
        # a1blk[db, 0] = W_enc^T rows db-block; a1blk[db, 1] = encT rows
        a1 = np.empty((NB, 2, P, H2), np.float32)
        a1[:, 0] = WT[H2:].reshape(NB, P, H2)
        a1[:, 1] = eT.reshape(NB, P, T_ENC)
        # blkA[db, p] = [W_dec^T row | decT row]
        bA = np.empty((NB, P, H2 + T_DEC), np.float32)
        bA[:, :, :H2] = WT[:H2].reshape(NB, P, H2)
        bA[:, :, H2:] = dT.reshape(NB, P, T_DEC)
        in_maps.append(
            {
                "encN": e.astype(bf),
                "a1blk": a1.astype(bf),
                "blkA": bA.astype(bf),
                "b4": b4,
            }
        )
    return in_maps


def kernel(encoderOutput, decoderInput, W, b, _trace=False):
    if "nc" not in _CACHE:
        _CACHE["nc"] = _build_nc()
    nc = _CACHE["nc"]
    in_maps = _prep_in_maps(encoderOutput, decoderInput, W, b)
    res = run_bass_kernel_spmd(nc, in_maps, core_ids=list(range(B)), trace=_trace)
    outs = np.stack([np.asarray(r["out"], np.float32).T for r in res.results])
    if _trace:
        _CACHE["last_result"] = res
    return outs

